# revision 32
# baseline (speedup 1.0000x reference)
"""Fused decoder attention block (self-attn + cross-attn + MLP) on 8 TRN2 NeuronCores.

Sharding: data-parallel over batch (B=16 -> 2 per core). No collectives.
v3 schedule: feature-major residual xT [D, n_tok]; q/k staged through DRAM
with contiguous tiles (x64 scale kept; 1/4096 folded into the softmax exp
scale); V kept in SBUF as fp8 (x2) with a 0.5-ones column so the PV matmul
runs fp8 DoubleRow over two s-chunks at a time and yields the denominator for
free; exp ops batched to [128,1024] over 2-bank PSUM score tiles; softmax
denominators batched into one [16,512] reciprocal_approx_fast per attention
phase; quickgelu via its exact tanh identity (x*sigmoid(1.702x) ==
(1+tanh(.851x))*(x/2)) so the MLP shares the exp_and_others ACT table with
attention (no table churn while zipped); LN stats (sum-x / sum-x^2) issued as
col-tiled concurrent matmuls into one PSUM bank.

Self-contained: hardcodes all shapes; only imports the system bass stack.
"""
import sys

sys.path.insert(0, "/opt/trn_rl_repo")

import numpy as np
import ml_dtypes

import concourse.tile as tile
from concourse import bacc, mybir
from concourse import bass_utils

F32 = mybir.dt.float32
BF16 = mybir.dt.bfloat16
F8 = mybir.dt.float8e4
AF = mybir.ActivationFunctionType
ALU = mybir.AluOpType
DR = mybir.MatmulPerfMode.DoubleRow
BF16NP = ml_dtypes.bfloat16
F8NP = ml_dtypes.float8_e4m3fn
WSC = 64.0                   # fp8 weight scale (host multiplies, drain divides)
IWSC = 1.0 / WSC
EXP_SC = 1.0 / (WSC * WSC)   # q,k both carry x64 -> scores carry x4096
WSC_V = 0.5                  # v_sb carries x0.5 (keeps |0.5*num| << f8 max)
ONESV = 1.0 / WSC_V          # ones column value -> denom row = 0.5*sum(e)
RNORM = ONESV / WSC_V        # post-reciprocal scale: cs*rI*RNORM = num/den

D = 1024
H = 16
HD = 64
T = 512
S = 1024
B = 16
NCORES = 8
BPC = B // NCORES            # batches per core = 2
N = T * BPC                  # x tokens per core = 1024
M = S * BPC                  # hidden tokens per core = 2048
DFF = 4 * D
KT = D // 128                # 8 k-tiles over D
EPS = 1e-5
GELU_A = 1.702
VS = 80                      # padded v_sb innermost stride (>=65, %16==0)


def _drive_until(primary, *fillers):
    """Round-robin emission; returns when `primary` is exhausted.
    Fillers keep their progress (pass the same generator to later phases)."""
    live = [f for f in fillers if f is not None]
    while True:
        try:
            next(primary)
        except StopIteration:
            return
        nxt = []
        for f in live:
            try:
                next(f)
                nxt.append(f)
            except StopIteration:
                pass
        live = nxt


def _drain(*gens):
    for g in gens:
        if g is None:
            continue
        for _ in g:
            pass


def _slow(g, k):
    """Wrap generator g so only every k-th advance steps it (filler pacing)."""
    while True:
        for _ in range(k - 1):
            yield
        try:
            next(g)
        except StopIteration:
            return
        yield


def build_program():
    nc = bacc.Bacc("TRN2", target_bir_lowering=False, debug=False,
                   enable_asserts=False, num_devices=NCORES)

    def din(name, shape, dt=BF16):
        return nc.dram_tensor(name, shape, dt, kind="ExternalInput").ap()

    xT_d = din("xT", [128, KT, N], F32)
    hT_d = din("hT", [128, KT, M], F8)
    wqk_d = din("wqk", [128, 16, KT, 128], F8)    # q:0-7, k:8-15
    wvsa_d = din("wvsa", [128, KT, D], F8)        # rhs layout for token-major V
    wosa_d = din("wosa", [128, 8, KT, 128], F8)
    wqca_d = din("wqca", [128, 8, KT, 128], F8)
    wkca_d = din("wkca", [128, 8, KT, 128], F8)
    wvca_d = din("wvca", [128, KT, D], F8)
    wfc_d = din("wfc", [128, 32, KT, 128])        # bf16, x0.5 (tanh-gelu)
    wproj_d = din("wproj", [128, 8, 32, 128])     # bf16
    wo_ca_d = din("woca", [128, 8, KT, 128], F8)
    sel_d = din("sel", [16, 8, 128], BF16)        # one-hot head-pair selector
    outT_d = nc.dram_tensor("outT", [128, KT, N], F32,
                            kind="ExternalOutput").ap()

    from contextlib import ExitStack
    with tile.TileContext(nc) as tc, ExitStack() as ctx:
        po = {}
        po["res"] = ctx.enter_context(tc.tile_pool(name="res", bufs=1))
        po["w"] = ctx.enter_context(tc.tile_pool(name="w", bufs=3))
        po["wb"] = ctx.enter_context(tc.tile_pool(name="wb", bufs=2))
        po["small"] = ctx.enter_context(tc.tile_pool(name="small", bufs=1))
        po["work"] = ctx.enter_context(tc.tile_pool(name="work", bufs=2))
        po["stg"] = ctx.enter_context(tc.tile_pool(name="stg", bufs=2))
        po["strm"] = ctx.enter_context(tc.tile_pool(name="strm", bufs=2))
        po["e8"] = ctx.enter_context(tc.tile_pool(name="e8", bufs=3))
        po["csb"] = ctx.enter_context(tc.tile_pool(name="csb", bufs=8))
        po["att"] = ctx.enter_context(tc.tile_pool(name="att", bufs=1))
        po["dram"] = ctx.enter_context(
            tc.tile_pool(name="dram", bufs=1, space="DRAM"))
        po["psum_pr"] = ctx.enter_context(
            tc.tile_pool(name="psum_pr", bufs=2, space="PSUM"))
        po["psum_sc"] = ctx.enter_context(
            tc.tile_pool(name="psum_sc", bufs=2, space="PSUM"))
        po["psum_ctx"] = ctx.enter_context(
            tc.tile_pool(name="psum_ctx", bufs=2, space="PSUM"))

        ones32 = po["res"].tile([128, 1], BF16, tag="ones")
        nc.vector.memset(ones32[:], 1.0)

        # ---- persistent SBUF state --------------------------------------
        xbuf = po["res"].tile([128, KT, N], F32, tag="xbuf")     # residual
        hbuf = po["res"].tile([128, KT, 512], BF16, tag="hbuf")  # LN3 out bf16
        h8 = po["res"].tile([128, KT, N], F8, tag="h8")          # LN1/2 out f8
        ctxT = po["res"].tile([128, 8, N], F8, tag="ctxT")       # attn output
        # v: [dv-in-sub(128), head, sub(16), 64 dv + ones(=0.5), pad to 80]
        v_sb = po["res"].tile([128, H, 16, VS], F8, tag="v_sb")
        gbuf = po["res"].tile([128, 32, 512], BF16, tag="gbuf")  # MLP hidden

        nc.vector.memset(v_sb[:, :, :, 64:65], ONESV)
        sel_sb = po["res"].tile([16, 8, 128], BF16, tag="sel")
        nc.sync.dma_start(sel_sb[:], sel_d[:])

        # per-kt loads so LN1's first stats matmul starts after ~1/8 of the load
        for ch in range(N // 512):
            sl = slice(ch * 512, (ch + 1) * 512)
            for kt in range(KT):
                nc.sync.dma_start(xbuf[:, kt, sl], xT_d[:, kt, sl])

        # DRAM scratch for q/k (contiguous tiles both ways)
        q_s = po["dram"].tile([128, 8, N], BF16, tag="q_s")      # self q
        q_c = po["dram"].tile([128, 8, N], BF16, tag="q_c")      # cross q
        k_s = po["dram"].tile([128, 8, N], BF16, tag="k_s")      # self k
        k_c = po["dram"].tile([128, 8, M], BF16, tag="k_c")      # cross k

        # ---- LayerNorm (generator; yields between sub-steps) ------------
        def gen_ln(tok_sl, to_f8, flag=None):
            """LN of xbuf[:, :, tok_sl] (512 tokens) -> h8[:, :, tok_sl] (f8)
            or hbuf[:, :, 0:512] (bf16, MLP input slot). Casts all of x/x^2
            up-front so the stats PSUM slot is held only for a short dense
            matmul burst (keeps the shared 'proj' slot free for fillers).
            Sets flag["stats_emitted"] once the shared xball/x2all tiles are
            consumed (gates the next LN's casts)."""
            t0 = tok_sl.start
            sl = slice(t0, t0 + 512)
            xball = po["work"].tile([128, KT, 512], BF16, tag="xball",
                                    bufs=1)
            x2all = po["work"].tile([128, KT, 512], BF16, tag="x2all",
                                    bufs=1)
            for k2 in range(KT // 2):
                ksl = slice(2 * k2, 2 * k2 + 2)
                nc.vector.tensor_copy(xball[:, ksl, :], xbuf[:, ksl, sl])
                nc.vector.tensor_tensor(x2all[:, ksl, :], xball[:, ksl, :],
                                        xball[:, ksl, :], ALU.mult)
                yield
            ps = po["psum_pr"].tile([128, 512], F32, tag="proj")
            for kt in range(KT):
                nc.tensor.matmul(ps[0:1, :], ones32[:], xball[:, kt, :],
                                 start=(kt == 0), stop=(kt == KT - 1),
                                 tile_position=(0, 0))
                nc.tensor.matmul(ps[32:33, :], ones32[:], x2all[:, kt, :],
                                 start=(kt == 0), stop=(kt == KT - 1),
                                 tile_position=(0, 32))
            if flag is not None:
                flag["stats_emitted"] = True
            sq2 = po["small"].tile([33, 512], F32, tag="sq2")
            nc.vector.tensor_copy(sq2[32:33, :], ps[32:33, :])
            var = po["small"].tile([1, 512], F32, tag="var")
            nc.gpsimd.dma_start(var[:], sq2[32:33, :])
            m = po["small"].tile([1, 512], F32, tag="m")
            nc.vector.tensor_scalar_mul(m[:], ps[0:1, :], 1.0 / D)
            a_b = po["small"].tile([128, 512], BF16, tag="Ab")
            b_b = po["small"].tile([128, 512], BF16, tag="Bb")
            mm = a_b[0:1, :]            # bf16 scratch for m^2 (tiny vs E[x^2])
            nc.vector.scalar_tensor_tensor(mm, m[:], 1.0, m[:],
                                           ALU.mult, ALU.mult)
            nc.vector.scalar_tensor_tensor(var[:], var[:], 1.0 / D,
                                           mm, ALU.mult, ALU.subtract)
            nc.vector.tensor_scalar_add(var[:], var[:], EPS)
            nc.scalar.activation(var[:], var[:], AF.Ln, bias=0.0)
            rstd16 = po["small"].tile([1, 512], BF16, tag="rstd16")
            nc.scalar.activation(rstd16[:], var[:], AF.Exp, scale=-0.5)
            nmrs16 = po["small"].tile([1, 512], BF16, tag="nmrs16")
            nc.vector.scalar_tensor_tensor(nmrs16[:], m[:], -1.0, rstd16[:],
                                           ALU.mult, ALU.mult)
            nc.gpsimd.partition_broadcast(a_b[:], rstd16[0:1, :])
            nc.gpsimd.partition_broadcast(b_b[:], nmrs16[0:1, :])
            yield
            if to_f8:
                dst = h8[:, :, sl]
            else:
                dst = hbuf[:, :, 0:512]
            ab3 = a_b[:].unsqueeze(1).broadcast_to([128, 2, 512])
            bb3 = b_b[:].unsqueeze(1).broadcast_to([128, 2, 512])
            for k2 in range(KT // 2):
                ksl = slice(2 * k2, 2 * k2 + 2)
                nc.vector.tensor_tensor(dst[:, ksl, :], xbuf[:, ksl, sl],
                                        ab3, ALU.mult)
                nc.vector.tensor_tensor(dst[:, ksl, :], dst[:, ksl, :],
                                        bb3, ALU.add)
                yield

        # ---- feature-major projection (generator) -----------------------
        def gen_fm_proj(w_ap, n_ot, kt_count, rhs3, tok_sl, out_cb, wtag,
                        pool="w", dr=False, wchunk=None):
            """for ot: psum[128,512] = sum_kt W[:,ot,kt].T @ rhs3[:,kt,tok_sl].
            dr=True: fp8 DoubleRow — two k-tiles per matmul.
            wchunk: k-tiles per weight DMA (default all)."""
            wdt = F8 if dr else BF16
            if wchunk is None:
                wchunk = kt_count
            for ot in range(n_ot):
                ps = po["psum_pr"].tile([128, 512], F32, tag="proj")
                for w0 in range(0, kt_count, wchunk):
                    wst = po[pool].tile([128, wchunk, 128], wdt, tag=wtag)
                    nc.sync.dma_start(wst[:], w_ap[:, ot, w0:w0 + wchunk])
                    if dr:
                        for k2 in range(wchunk // 2):
                            kk = w0 + 2 * k2
                            nc.tensor.matmul(
                                ps[:], wst[:, 2 * k2:2 * k2 + 2, :],
                                rhs3[:, kk:kk + 2, tok_sl],
                                start=(kk == 0),
                                stop=(kk == kt_count - 2),
                                perf_mode=DR)
                            if k2 == wchunk // 4:
                                yield
                    else:
                        for k in range(wchunk):
                            kk = w0 + k
                            nc.tensor.matmul(ps[:], wst[:, k],
                                             rhs3[:, kk, tok_sl],
                                             start=(kk == 0),
                                             stop=(kk == kt_count - 1))
                            if k == wchunk // 2:
                                yield
                out_cb(ot, ps)
                yield "ot"

        def stage_to_dram(ps, dram_ap):
            # scalar-engine copy: ACT Copy is in every table set and the
            # scalar engine is idle during the projection-heavy phases
            stg = po["stg"].tile([128, 512], BF16, tag="stg")
            nc.scalar.copy(stg[:], ps[:])
            nc.sync.dma_start(dram_ap, stg[:])

        # ---- token-major V projection (generator) -----------------------
        def gen_v_proj(h3, wv_d, sub0, tok0):
            """V proj (fp8 DoubleRow) for 512 tokens [tok0, tok0+512) of h3
            -> v_sb subs sub0..sub0+3 (f8, x WSC_V)."""
            for ch in range(2):           # dv chunks of 512 = 8 heads
                wvc = po["wb"].tile([128, KT, 512], F8, tag="wbigq")
                nc.sync.dma_start(wvc[:], wv_d[:, :, ch * 512:(ch + 1) * 512])
                for tt in range(4):
                    tsl = slice(tok0 + tt * 128, tok0 + (tt + 1) * 128)
                    ps = po["psum_pr"].tile([128, 512], F32, tag="proj")
                    for k2 in range(KT // 2):
                        nc.tensor.matmul(
                            ps[:], h3[:, 2 * k2:2 * k2 + 2, tsl],
                            wvc[:, 2 * k2:2 * k2 + 2, :],
                            start=(k2 == 0), stop=(k2 == KT // 2 - 1),
                            perf_mode=DR)
                        if k2 == KT // 4:
                            yield
                    sub = sub0 + tt
                    nc.vector.tensor_copy(
                        v_sb[:, ch * 8:(ch + 1) * 8, sub, 0:64],
                        ps[:].rearrange("p (h e) -> p h e", e=64))
                    yield

        # ---- cross-attn K projection (generator, from hT stream) --------
        def gen_ca_k():
            for hch in range(M // 512):
                hsl = slice(hch * 512, (hch + 1) * 512)
                hTc = po["strm"].tile([128, KT, 512], F8, tag="hTc")
                nc.sync.dma_start(hTc[:], hT_d[:, :, hsl])
                for ot in range(8):
                    wst = po["w"].tile([128, KT, 128], F8, tag="wst8q")
                    nc.sync.dma_start(wst[:], wkca_d[:, ot])
                    ps = po["psum_pr"].tile([128, 512], F32, tag="proj")
                    for k2 in range(KT // 2):
                        nc.tensor.matmul(
                            ps[:], wst[:, 2 * k2:2 * k2 + 2, :],
                            hTc[:, 2 * k2:2 * k2 + 2, :],
                            start=(k2 == 0), stop=(k2 == KT // 2 - 1),
                            perf_mode=DR)
                        if k2 == 1:
                            yield
                    stage_to_dram(ps, k_c[:, ot, hsl])
                    yield

        # ---- cross-attn V projection (generator, from hT stream) --------
        def gen_ca_v(b, sub0):
            for hch in range(2):          # two 512-token chunks per batch
                tok0 = b * S + hch * 512
                hsl = slice(tok0, tok0 + 512)
                hTc = po["strm"].tile([128, KT, 512], F8, tag="hTc")
                nc.sync.dma_start(hTc[:], hT_d[:, :, hsl])
                yield from gen_v_proj(hTc, wvca_d, sub0 + 4 * hch, 0)

        # ---- attention (generator) --------------------------------------
        def gen_attention(q_dr, k_dr, sub0, s_len, b, st):
            """Attention for batch b: q/k strips from DRAM, v from v_sb subs
            [sub0, sub0 + s_len/128). Scores e/o row-paired; exp [128,1024]
            f32->f8; PV fp8 DoubleRow over 2 s-chunks. Unnormalized ctx (f8)
            and denominators are collected into `st`; normalization happens
            in gen_att_norm (scheduled as a filler of the next phase)."""
            n_s = s_len // 128
            bsl = slice(b * T, (b + 1) * T)
            rD = po["att"].tile([16, 512], BF16, tag="rD", bufs=2)
            cs = []
            st["rD"] = rD
            st["cs"] = cs
            for hp in range(H // 2):
                qp = po["strm"].tile([128, 512], BF16, tag="qp")
                nc.sync.dma_start(qp[:], q_dr[:, hp, bsl])
                kp = po["strm"].tile([128, 1024], BF16, tag="kp")
                nc.sync.dma_start(kp[:, 0:s_len],
                                  k_dr[:, hp, b * s_len:(b + 1) * s_len])
                ctx_e = po["psum_ctx"].tile([65, 512], F32, tag="ctx")
                ctx_o = po["psum_ctx"].tile([65, 512], F32, tag="ctx")
                h0 = hp * 2
                for c2 in range(n_s // 2):
                    sc_e = po["psum_sc"].tile([128, 1024], F32, tag="sc")
                    sc_o = po["psum_sc"].tile([128, 1024], F32, tag="sc")
                    for j in range(2):
                        ssl = slice((2 * c2 + j) * 128, (2 * c2 + j + 1) * 128)
                        osl = slice(j * 512, (j + 1) * 512)
                        # paired: rows 0-63 and 64-127 run concurrently
                        nc.tensor.matmul(sc_e[:, osl], kp[0:64, ssl],
                                         qp[0:64, :], start=True, stop=True)
                        nc.tensor.matmul(sc_o[:, osl], kp[64:128, ssl],
                                         qp[64:128, :], start=True, stop=True)
                    e_e = po["e8"].tile([128, 2, 512], F8, tag="e")
                    e_o = po["e8"].tile([128, 2, 512], F8, tag="e")
                    nc.scalar.activation(
                        e_e[:].rearrange("p a t -> p (a t)"),
                        sc_e[:], AF.Exp, scale=EXP_SC)
                    nc.scalar.activation(
                        e_o[:].rearrange("p a t -> p (a t)"),
                        sc_o[:], AF.Exp, scale=EXP_SC)
                    yield
                    st = (c2 == 0)
                    sp = (c2 == n_s // 2 - 1)
                    sub = sub0 + 2 * c2
                    nc.tensor.matmul(
                        ctx_e[:], v_sb[:, h0, sub:sub + 2, 0:65],
                        e_e[:], start=st, stop=sp, perf_mode=DR)
                    nc.tensor.matmul(
                        ctx_o[:], v_sb[:, h0 + 1, sub:sub + 2, 0:65],
                        e_o[:], start=st, stop=sp, perf_mode=DR)
                    yield
                # epilogue: drain unnormalized ctx (f8, x WSC_V) + denom rows
                # (bf16) so the psum banks free quickly; normalization happens
                # after the batched reciprocal below.
                cs_e = po["csb"].tile([64, 512], F8, tag="cse")
                cs_o = po["csb"].tile([64, 512], F8, tag="cso")
                dn = po["work"].tile([65, 512], BF16, tag="dn")
                nc.vector.tensor_copy(cs_e[:], ctx_e[0:64, :])
                nc.vector.tensor_copy(dn[64:65, :], ctx_e[64:65, :])
                nc.gpsimd.dma_start(rD[2 * hp:2 * hp + 1, :], dn[64:65, :])
                yield
                dn2 = po["work"].tile([65, 512], BF16, tag="dn")
                nc.vector.tensor_copy(cs_o[:], ctx_o[0:64, :])
                nc.vector.tensor_copy(dn2[64:65, :], ctx_o[64:65, :])
                nc.gpsimd.dma_start(rD[2 * hp + 1:2 * hp + 2, :],
                                    dn2[64:65, :])
                cs.append((cs_e, cs_o))
                yield

        def gen_att_norm(st, b):
            """Normalize collected ctx by the batched softmax reciprocals and
            write ctxT. Runs as a filler of the phase after the attention."""
            bsl = slice(b * T, (b + 1) * T)
            rD = st["rD"]
            cs = st["cs"]
            rDf = po["att"].tile([16, 512], F32, tag="rDf")
            nc.vector.tensor_copy(rDf[:], rD[:])
            rI = po["att"].tile([16, 512], F32, tag="rI")
            nc.vector.reciprocal_approx_fast(rI[:], rDf[:])
            rI16 = po["att"].tile([16, 512], BF16, tag="rI16")
            nc.vector.tensor_scalar_mul(rI16[:], rI[:], RNORM)
            yield
            for hp in range(H // 2):
                cs_e, cs_o = cs[hp]
                # broadcast the pair's reciprocals across partitions with a
                # rank-16 PE matmul: rows 0-63 <- rI16[2hp], 64-127 <- [2hp+1]
                rb = po["psum_sc"].tile([128, 1024], F32, tag="sc")
                nc.tensor.matmul(rb[:, 0:512], sel_sb[:, hp, :], rI16[:],
                                 start=True, stop=True)
                nc.vector.tensor_tensor(ctxT[0:64, hp, bsl], cs_e[:],
                                        rb[0:64, 0:512], ALU.mult)
                yield
                todd = po["work"].tile([64, 512], F8, tag="todd")
                nc.vector.tensor_tensor(todd[:], cs_o[:], rb[64:128, 0:512],
                                        ALU.mult)
                nc.gpsimd.dma_start(ctxT[64:128, hp, bsl], todd[:])
                yield

        # ---- out-projection (generator) ---------------------------------
        def gen_out_proj(w_d, b):
            tsl = slice(b * 512, (b + 1) * 512)

            def cb(ot, ps, _tsl=tsl):
                nc.vector.scalar_tensor_tensor(
                    xbuf[:, ot, _tsl], ps[:], IWSC, xbuf[:, ot, _tsl],
                    ALU.mult, ALU.add)
            yield from gen_fm_proj(w_d, 8, KT, ctxT, tsl, cb, "wst8q",
                                   dr=True)

        # ---- qkv for self-attention (generator) -------------------------
        def gen_sa_qkv():
            for bch in range(2):
                tsl = slice(bch * 512, (bch + 1) * 512)

                def qk_cb(ot, ps, _tsl=tsl):
                    if ot < 8:
                        stage_to_dram(ps, q_s[:, ot, _tsl])
                    else:
                        stage_to_dram(ps, k_s[:, ot - 8, _tsl])
                yield from gen_fm_proj(wqk_d, 16, KT, h8, tsl, qk_cb,
                                       "wst8q", dr=True)
            for b in range(2):
                yield from gen_v_proj(h8, wvsa_d, 4 * b, b * 512)

        # ---- cross-attn q projection (generator) ------------------------
        def gen_ca_q(b):
            tsl = slice(b * 512, (b + 1) * 512)

            def q2_cb(ot, ps, _tsl=tsl):
                stage_to_dram(ps, q_c[:, ot, _tsl])
            yield from gen_fm_proj(wqca_d, 8, KT, h8, tsl, q2_cb, "wst8q",
                                   dr=True)

        # ---- MLP (generator, one 512-token batch chunk) ------------------
        def gen_mlp(b):
            tsl = slice(b * 512, (b + 1) * 512)

            def fc_cb(ot, ps):
                # psum = fc_true/2 (wfc halved on host); quickgelu(x) ==
                # (1+tanh(0.851x)) * x/2, and tanh lives in exp_and_others.
                th = po["work"].tile([128, 512], BF16, tag="th")
                nc.scalar.activation(th[:], ps[:], AF.Tanh, scale=GELU_A)
                nc.vector.scalar_tensor_tensor(gbuf[:, ot], th[:], 1.0,
                                               ps[:], ALU.add, ALU.mult)
            yield from gen_fm_proj(wfc_d, 32, KT, hbuf, slice(0, 512),
                                   fc_cb, "wst16")

            def proj_cb(ot, ps, _tsl=tsl):
                nc.vector.tensor_tensor(xbuf[:, ot, _tsl], ps[:],
                                        xbuf[:, ot, _tsl], ALU.add)
                nc.sync.dma_start(outT_d[:, ot, _tsl], xbuf[:, ot, _tsl])
            yield from gen_fm_proj(wproj_d, 8, 32, gbuf, slice(0, 512),
                                   proj_cb, "wbig", pool="wb", wchunk=16)

        # =================== schedule ====================================
        cak = gen_ca_k()

        # P0: LN1 zipped with cross-K (independent, fills the LN ramp)
        _drive_until(gen_ln(slice(0, 512), True), cak)
        _drive_until(gen_ln(slice(512, 1024), True), cak)

        # P1: SA qkv (dense; keep cak for the attention phases)
        _drive_until(gen_sa_qkv())

        # P2: SA attention b0; zipped with cak + CA-V(b0) into subs 8-15
        cav0 = gen_ca_v(0, 8)
        st_s0, st_s1, st_c0, st_c1 = {}, {}, {}, {}
        _drive_until(gen_attention(q_s, k_s, 0, T, 0, st_s0), cav0, cak)

        # P3: SA attention b1; zipped with
        # norm(SA b0)+SAout(b0)+LN2(b0)+CAq(b0) + rest
        def gen_tail0():
            yield from gen_att_norm(st_s0, 0)
            yield from gen_out_proj(wosa_d, 0)
            yield from gen_ln(slice(0, 512), True)
            yield from gen_ca_q(0)
        tail0 = gen_tail0()
        _drive_until(gen_attention(q_s, k_s, 4, T, 1, st_s1), tail0, cav0,
                     cak)

        # P4: CA attention b0 (subs 8-15); zipped with
        # norm(SA b1)+SAout(b1)+LN2(b1)+CAq(b1) and CA-V(b1) into subs 0-7
        _drain(tail0, cav0, cak)

        def gen_tail1():
            yield from gen_att_norm(st_s1, 1)
            yield from gen_out_proj(wosa_d, 1)
            yield from gen_ln(slice(512, 1024), True)
            yield from gen_ca_q(1)
        tail1 = gen_tail1()
        cav1 = gen_ca_v(1, 0)
        _drive_until(gen_attention(q_c, k_c, 8, S, 0, st_c0), tail1, cav1)

        # P5: CA attention b1 (subs 0-7); zipped with
        # norm(CA b0)+CAout(b0)+LN3(b0)+MLP(b0)
        _drain(tail1, cav1)

        ln3b0_flag = {"stats_emitted": False}

        def gen_tail2():
            yield from gen_att_norm(st_c0, 0)
            yield from gen_out_proj(wo_ca_d, 0)
            yield from gen_ln(slice(0, 512), False, ln3b0_flag)
            yield from gen_mlp(0)
        tail2 = gen_tail2()
        _drive_until(gen_attention(q_c, k_c, 0, S, 1, st_c1), _slow(tail2, 2))

        # P6: finish MLP(b0), zipped with norm(CA b1)+CAout(b1), then emit
        # LN3(b1)'s casts+stats+rstd as soon as LN3(b0)'s stats have consumed
        # the shared xball tiles (the hbuf/gbuf-touching LN3 apply + MLP(b1)
        # wait for P7).
        ln3b = gen_ln(slice(512, 1024), False)

        def gen_tail3a():
            yield from gen_att_norm(st_c1, 1)
            yield from gen_out_proj(wo_ca_d, 1)
            while not ln3b0_flag["stats_emitted"]:
                yield                   # tail2 (the primary) will get there
            for _ in range(KT // 2 + 1):
                next(ln3b)              # casts + dense stats + rstd chain
                yield
        tail3a = gen_tail3a()
        _drive_until(tail2, tail3a)
        _drain(tail3a)

        # P7: LN3(b1) apply + MLP(b1)  (dense tail)
        def gen_tail3b():
            yield from ln3b
            yield from gen_mlp(1)
        _drive_until(gen_tail3b())

    nc.compile()
    return nc


# ---------------------------------------------------------------------------
# host side
# ---------------------------------------------------------------------------

def _tile4(w):
    """[Din, Dout] -> [128, Dout/128, Din/128, 128] (p, ot, kt, o)."""
    din, dout = w.shape
    return np.ascontiguousarray(
        w.reshape(din // 128, 128, dout // 128, 128).transpose(1, 2, 0, 3))


def _rhs_tiled(w):
    """[Din, Dout] -> [128, Din/128, Dout] (p, kt, o)."""
    din, dout = w.shape
    return np.ascontiguousarray(
        w.reshape(din // 128, 128, dout).transpose(1, 0, 2))


def _prep_host(inputs):
    f32 = np.float32
    g = {k: np.asarray(v, f32) for k, v in inputs.items()}
    x, hs = g["x"], g["hidden_states"]
    scale = f32(1.0 / np.sqrt(HD))

    wq, wk, wv = np.split(g["sa_in_w"], 3, axis=0)
    bq, bk, bv = np.split(g["sa_in_b"], 3)
    wq_e = (wq * g["ln1_g"][None, :]) * scale
    bq_e = (wq @ g["ln1_b"]) * scale + bq
    wk_e = wk * g["ln1_g"][None, :]
    bk_e = wk @ g["ln1_b"] + bk
    wv_e = wv * g["ln1_g"][None, :]
    bv_e = wv @ g["ln1_b"] + bv

    cq, ck, cv = np.split(g["ca_in_w"], 3, axis=0)
    cbq, cbk, cbv = np.split(g["ca_in_b"], 3)
    cq_e = (cq * g["ln2_g"][None, :]) * scale
    cbq_e = (cq @ g["ln2_b"]) * scale + cbq
    # k/v of cross-attn apply to raw hidden_states: no LN fold
    fc_e = g["fc_w"] * g["ln3_g"][None, :]
    fcb_e = g["fc_w"] @ g["ln3_b"] + g["fc_b"]

    nz = lambda a: bool(np.abs(a).max() > 0)
    assert not any(nz(a) for a in
                   (bq_e, bk_e, bv_e, g["sa_out_b"], cbq_e, cbk, cbv,
                    g["ca_out_b"], fcb_e, g["proj_b"])), \
        "kernel compiled for the zero-bias configuration"

    wqk = np.concatenate([wq_e, wk_e], axis=0)     # [2D, D]

    bf = lambda a: np.ascontiguousarray(a.astype(BF16NP))
    f8 = lambda a, s=WSC: np.ascontiguousarray(
        (a * np.float32(s)).astype(F8NP))
    weights = {
        "wqk": f8(_tile4(wqk.T)),
        "wvsa": f8(_rhs_tiled(wv_e.T), WSC_V),
        "wosa": f8(_tile4(g["sa_out_w"].T)),
        "wqca": f8(_tile4(cq_e.T)),
        "wkca": f8(_tile4(ck.T)),
        "wvca": f8(_rhs_tiled(cv.T), WSC_V),
        "woca": f8(_tile4(g["ca_out_w"].T)),
        "wfc": bf(_tile4(fc_e.T) * np.float32(0.5)),
        "wproj": bf(_tile4(g["proj_w"].T)),
    }
    sel = np.zeros((16, 8, 128), f32)
    for hp in range(8):
        sel[2 * hp, hp, 0:64] = 1.0
        sel[2 * hp + 1, hp, 64:128] = 1.0
    weights["sel"] = bf(sel)

    in_maps = []
    for c in range(NCORES):
        xs = x[:, 2 * c:2 * c + 2, :]              # [T, 2, D]
        xt = xs.transpose(2, 1, 0).reshape(KT, 128, N).transpose(1, 0, 2)
        hss = hs[:, 2 * c:2 * c + 2, :]
        ht = hss.transpose(2, 1, 0).reshape(KT, 128, M).transpose(1, 0, 2)
        im = dict(weights)
        im["xT"] = np.ascontiguousarray(xt.astype(f32))
        im["hT"] = np.ascontiguousarray(ht.astype(F8NP))
        in_maps.append(im)
    return in_maps


def _unshard(results):
    out = np.empty((T, B, D), np.float32)
    for c in range(NCORES):
        r = np.asarray(results[c]["outT"])         # [128, KT, N]
        arr = r.transpose(1, 0, 2).reshape(D, BPC, T)
        out[:, 2 * c:2 * c + 2, :] = arr.transpose(2, 1, 0)
    return out


_cache = {}


def _get_program():
    if "nc" not in _cache:
        _cache["nc"] = build_program()
    return _cache["nc"]


def kernel(**inputs):
    in_maps = _prep_host(inputs)
    nc = _get_program()
    res = bass_utils.run_bass_kernel_spmd(nc, in_maps,
                                          core_ids=list(range(NCORES)))
    return _unshard(res.results)


def kernel_traced(**inputs):
    """Like kernel() but with NTFF profiling; returns (out, exec_time_ns)."""
    import types
    import antenv  # noqa: F401
    if "antenv.axon_hooks" not in sys.modules:
        hooks = types.ModuleType("antenv.axon_hooks")
        hooks._hook = None
        hooks.set_axon_ntff_profile_hook = lambda h: setattr(hooks, "_hook", h)
        hooks.get_axon_ntff_profile_hook = lambda: hooks._hook
        sys.modules["antenv.axon_hooks"] = hooks
        try:
            import trn_agent_boot.trn_boot as _tb
            hooks._hook = _tb._ntff_profile_via_ctypes("/opt/axon/libaxon_pjrt.so")
        except Exception as e:  # pragma: no cover
            print("ntff hook unavailable:", e)
    in_maps = _prep_host(inputs)
    nc = _get_program()
    res = bass_utils.run_bass_kernel_spmd(nc, in_maps,
                                          core_ids=list(range(NCORES)),
                                          trace=True)
    return _unshard(res.results), res.exec_time_ns


# revision 39
# speedup vs baseline: 1.0500x; 1.0500x over previous
"""Fused decoder attention block (self-attn + cross-attn + MLP) on 8 TRN2 NeuronCores.

Sharding: data-parallel over batch (B=16 -> 2 per core). No collectives.
v3 schedule: feature-major residual xT [D, n_tok]; q/k staged through DRAM
with contiguous tiles (x64 scale kept; 1/4096 folded into the softmax exp
scale); V kept in SBUF as fp8 (x2) with a 0.5-ones column so the PV matmul
runs fp8 DoubleRow over two s-chunks at a time and yields the denominator for
free; exp ops batched to [128,1024] over 2-bank PSUM score tiles; softmax
denominators batched into one [16,512] reciprocal_approx_fast per attention
phase; quickgelu via its exact tanh identity (x*sigmoid(1.702x) ==
(1+tanh(.851x))*(x/2)) so the MLP shares the exp_and_others ACT table with
attention (no table churn while zipped); LN stats (sum-x / sum-x^2) issued as
col-tiled concurrent matmuls into one PSUM bank.

Self-contained: hardcodes all shapes; only imports the system bass stack.
"""
import sys

sys.path.insert(0, "/opt/trn_rl_repo")

import numpy as np
import ml_dtypes

import concourse.tile as tile
from concourse import bacc, mybir
from concourse import bass_utils

F32 = mybir.dt.float32
BF16 = mybir.dt.bfloat16
F8 = mybir.dt.float8e4
AF = mybir.ActivationFunctionType
ALU = mybir.AluOpType
DR = mybir.MatmulPerfMode.DoubleRow
BF16NP = ml_dtypes.bfloat16
F8NP = ml_dtypes.float8_e4m3fn
WSC = 64.0                   # fp8 weight scale (host multiplies, drain divides)
IWSC = 1.0 / WSC
EXP_SC = 1.0 / (WSC * WSC)   # q,k both carry x64 -> scores carry x4096
WSC_V = 0.5                  # v_sb carries x0.5 (keeps |0.5*num| << f8 max)
ONESV = 1.0 / WSC_V          # ones column value -> denom row = 0.5*sum(e)
RNORM = ONESV / WSC_V        # post-reciprocal scale: cs*rI*RNORM = num/den

D = 1024
H = 16
HD = 64
T = 512
S = 1024
B = 16
NCORES = 8
BPC = B // NCORES            # batches per core = 2
N = T * BPC                  # x tokens per core = 1024
M = S * BPC                  # hidden tokens per core = 2048
DFF = 4 * D
KT = D // 128                # 8 k-tiles over D
EPS = 1e-5
GELU_A = 1.702
VS = 80                      # padded v_sb innermost stride (>=65, %16==0)


def _drive_until(primary, *fillers):
    """Round-robin emission; returns when `primary` is exhausted.
    Fillers keep their progress (pass the same generator to later phases)."""
    live = [f for f in fillers if f is not None]
    while True:
        try:
            next(primary)
        except StopIteration:
            return
        nxt = []
        for f in live:
            try:
                next(f)
                nxt.append(f)
            except StopIteration:
                pass
        live = nxt


def _drain(*gens):
    for g in gens:
        if g is None:
            continue
        for _ in g:
            pass


def _slow(g, k):
    """Wrap generator g so only every k-th advance steps it (filler pacing)."""
    while True:
        for _ in range(k - 1):
            yield
        try:
            next(g)
        except StopIteration:
            return
        yield


def build_program():
    nc = bacc.Bacc("TRN2", target_bir_lowering=False, debug=False,
                   enable_asserts=False, num_devices=NCORES)

    def din(name, shape, dt=BF16):
        return nc.dram_tensor(name, shape, dt, kind="ExternalInput").ap()

    xT_d = din("xT", [128, KT, N], F32)
    hT_d = din("hT", [128, KT, M], F8)
    wqk_d = din("wqk", [128, 16, KT, 128], F8)    # q:0-7, k:8-15
    wvsa_d = din("wvsa", [128, KT, D], F8)        # rhs layout for token-major V
    wosa_d = din("wosa", [128, 8, KT, 128], F8)
    wqca_d = din("wqca", [128, 8, KT, 128], F8)
    wkca_d = din("wkca", [128, 8, KT, 128], F8)
    wvca_d = din("wvca", [128, KT, D], F8)
    wfc_d = din("wfc", [128, 32, KT, 128])        # bf16, x0.5 (tanh-gelu)
    wproj_d = din("wproj", [128, 8, 32, 128])     # bf16
    wo_ca_d = din("woca", [128, 8, KT, 128], F8)
    sel_d = din("sel", [16, 8, 128], BF16)        # one-hot head-pair selector
    outT_d = nc.dram_tensor("outT", [128, KT, N], F32,
                            kind="ExternalOutput").ap()

    from contextlib import ExitStack
    with tile.TileContext(nc) as tc, ExitStack() as ctx:
        po = {}
        po["res"] = ctx.enter_context(tc.tile_pool(name="res", bufs=1))
        po["w"] = ctx.enter_context(tc.tile_pool(name="w", bufs=3))
        po["wb"] = ctx.enter_context(tc.tile_pool(name="wb", bufs=2))
        po["small"] = ctx.enter_context(tc.tile_pool(name="small", bufs=1))
        po["work"] = ctx.enter_context(tc.tile_pool(name="work", bufs=2))
        po["stg"] = ctx.enter_context(tc.tile_pool(name="stg", bufs=2))
        po["strm"] = ctx.enter_context(tc.tile_pool(name="strm", bufs=2))
        po["e8"] = ctx.enter_context(tc.tile_pool(name="e8", bufs=3))
        po["csb"] = ctx.enter_context(tc.tile_pool(name="csb", bufs=8))
        po["att"] = ctx.enter_context(tc.tile_pool(name="att", bufs=1))
        po["dram"] = ctx.enter_context(
            tc.tile_pool(name="dram", bufs=1, space="DRAM"))
        po["psum_pr"] = ctx.enter_context(
            tc.tile_pool(name="psum_pr", bufs=2, space="PSUM"))
        po["psum_sc"] = ctx.enter_context(
            tc.tile_pool(name="psum_sc", bufs=2, space="PSUM"))
        po["psum_ctx"] = ctx.enter_context(
            tc.tile_pool(name="psum_ctx", bufs=2, space="PSUM"))

        ones32 = po["res"].tile([128, 1], BF16, tag="ones")
        nc.vector.memset(ones32[:], 1.0)

        # ---- persistent SBUF state --------------------------------------
        xbuf = po["res"].tile([128, KT, N], F32, tag="xbuf")     # residual
        hbuf = po["res"].tile([128, KT, 512], BF16, tag="hbuf")  # LN3 out bf16
        h8 = po["res"].tile([128, KT, N], F8, tag="h8")          # LN1/2 out f8
        ctxT = po["res"].tile([128, 8, N], F8, tag="ctxT")       # attn output
        # v: [dv-in-sub(128), head, sub(16), 64 dv + ones(=0.5), pad to 80]
        v_sb = po["res"].tile([128, H, 16, VS], F8, tag="v_sb")
        gbuf = po["res"].tile([128, 32, 512], BF16, tag="gbuf")  # MLP hidden

        nc.vector.memset(v_sb[:, :, :, 64:65], ONESV)
        sel_sb = po["res"].tile([16, 8, 128], BF16, tag="sel")
        nc.sync.dma_start(sel_sb[:], sel_d[:])

        # DRAM scratch for q/k (contiguous tiles both ways)
        q_s = po["dram"].tile([128, 8, N], BF16, tag="q_s")      # self q
        q_c = po["dram"].tile([128, 8, N], BF16, tag="q_c")      # cross q
        k_s = po["dram"].tile([128, 8, N], BF16, tag="k_s")      # self k
        k_c = po["dram"].tile([128, 8, M], BF16, tag="k_c")      # cross k

        # ---- LayerNorm (generator; yields between sub-steps) ------------
        def gen_ln(tok_sl, to_f8, flag=None):
            """LN of xbuf[:, :, tok_sl] (512 tokens) -> h8[:, :, tok_sl] (f8)
            or hbuf[:, :, 0:512] (bf16, MLP input slot). Casts all of x/x^2
            up-front so the stats PSUM slot is held only for a short dense
            matmul burst (keeps the shared 'proj' slot free for fillers).
            Sets flag["stats_emitted"] once the shared xball/x2all tiles are
            consumed (gates the next LN's casts)."""
            t0 = tok_sl.start
            sl = slice(t0, t0 + 512)
            xball = po["work"].tile([128, KT, 512], BF16, tag="xball",
                                    bufs=1)
            x2all = po["work"].tile([128, KT, 512], BF16, tag="x2all",
                                    bufs=1)
            for k2 in range(KT // 2):
                ksl = slice(2 * k2, 2 * k2 + 2)
                nc.vector.tensor_copy(xball[:, ksl, :], xbuf[:, ksl, sl])
                nc.vector.tensor_tensor(x2all[:, ksl, :], xball[:, ksl, :],
                                        xball[:, ksl, :], ALU.mult)
                yield
            ps = po["psum_pr"].tile([128, 512], F32, tag="proj")
            for kt in range(KT):
                nc.tensor.matmul(ps[0:1, :], ones32[:], xball[:, kt, :],
                                 start=(kt == 0), stop=(kt == KT - 1),
                                 tile_position=(0, 0))
                nc.tensor.matmul(ps[32:33, :], ones32[:], x2all[:, kt, :],
                                 start=(kt == 0), stop=(kt == KT - 1),
                                 tile_position=(0, 32))
            if flag is not None:
                flag["stats_emitted"] = True
            sq2 = po["small"].tile([33, 512], F32, tag="sq2")
            nc.vector.tensor_copy(sq2[32:33, :], ps[32:33, :])
            var = po["small"].tile([1, 512], F32, tag="var")
            nc.gpsimd.dma_start(var[:], sq2[32:33, :])
            m = po["small"].tile([1, 512], F32, tag="m")
            nc.vector.tensor_scalar_mul(m[:], ps[0:1, :], 1.0 / D)
            a_b = po["small"].tile([128, 512], BF16, tag="Ab")
            b_b = po["small"].tile([128, 512], BF16, tag="Bb")
            mm = a_b[0:1, :]            # bf16 scratch for m^2 (tiny vs E[x^2])
            nc.vector.scalar_tensor_tensor(mm, m[:], 1.0, m[:],
                                           ALU.mult, ALU.mult)
            nc.vector.scalar_tensor_tensor(var[:], var[:], 1.0 / D,
                                           mm, ALU.mult, ALU.subtract)
            nc.vector.tensor_scalar_add(var[:], var[:], EPS)
            nc.scalar.activation(var[:], var[:], AF.Ln, bias=0.0)
            rstd16 = po["small"].tile([1, 512], BF16, tag="rstd16")
            nc.scalar.activation(rstd16[:], var[:], AF.Exp, scale=-0.5)
            nmrs16 = po["small"].tile([1, 512], BF16, tag="nmrs16")
            nc.vector.scalar_tensor_tensor(nmrs16[:], m[:], -1.0, rstd16[:],
                                           ALU.mult, ALU.mult)
            nc.gpsimd.partition_broadcast(a_b[:], rstd16[0:1, :])
            nc.gpsimd.partition_broadcast(b_b[:], nmrs16[0:1, :])
            yield
            if to_f8:
                dst = h8[:, :, sl]
            else:
                dst = hbuf[:, :, 0:512]
            ab3 = a_b[:].unsqueeze(1).broadcast_to([128, 2, 512])
            bb3 = b_b[:].unsqueeze(1).broadcast_to([128, 2, 512])
            for k2 in range(KT // 2):
                ksl = slice(2 * k2, 2 * k2 + 2)
                nc.vector.tensor_tensor(dst[:, ksl, :], xbuf[:, ksl, sl],
                                        ab3, ALU.mult)
                nc.vector.tensor_tensor(dst[:, ksl, :], dst[:, ksl, :],
                                        bb3, ALU.add)
                yield

        # ---- feature-major projection (generator) -----------------------
        def gen_fm_proj(w_ap, n_ot, kt_count, rhs3, tok_sl, out_cb, wtag,
                        pool="w", dr=False, wchunk=None, psp="psum_pr",
                        pst="proj"):
            """for ot: psum[128,512] = sum_kt W[:,ot,kt].T @ rhs3[:,kt,tok_sl].
            dr=True: fp8 DoubleRow — two k-tiles per matmul.
            wchunk: k-tiles per weight DMA (default all)."""
            wdt = F8 if dr else BF16
            if wchunk is None:
                wchunk = kt_count
            for ot in range(n_ot):
                ps = po[psp].tile([128, 512], F32, tag=pst)
                for w0 in range(0, kt_count, wchunk):
                    wst = po[pool].tile([128, wchunk, 128], wdt, tag=wtag)
                    nc.sync.dma_start(wst[:], w_ap[:, ot, w0:w0 + wchunk])
                    if dr:
                        for k2 in range(wchunk // 2):
                            kk = w0 + 2 * k2
                            nc.tensor.matmul(
                                ps[:], wst[:, 2 * k2:2 * k2 + 2, :],
                                rhs3[:, kk:kk + 2, tok_sl],
                                start=(kk == 0),
                                stop=(kk == kt_count - 2),
                                perf_mode=DR)
                            if k2 == wchunk // 4:
                                yield
                    else:
                        for k in range(wchunk):
                            kk = w0 + k
                            nc.tensor.matmul(ps[:], wst[:, k],
                                             rhs3[:, kk, tok_sl],
                                             start=(kk == 0),
                                             stop=(kk == kt_count - 1))
                            if k == wchunk // 2:
                                yield
                out_cb(ot, ps)
                yield "ot"

        def stage_to_dram(ps, dram_ap):
            # scalar-engine copy: ACT Copy is in every table set and the
            # scalar engine is idle during the projection-heavy phases
            stg = po["stg"].tile([128, 512], BF16, tag="stg")
            nc.scalar.copy(stg[:], ps[:])
            nc.sync.dma_start(dram_ap, stg[:])

        # ---- token-major V projection (generator) -----------------------
        def gen_v_proj(h3, wv_d, sub0, tok0):
            """V proj (fp8 DoubleRow) for 512 tokens [tok0, tok0+512) of h3
            -> v_sb subs sub0..sub0+3 (f8, x WSC_V)."""
            for ch in range(2):           # dv chunks of 512 = 8 heads
                wvc = po["wb"].tile([128, KT, 512], F8, tag="wbigq")
                nc.sync.dma_start(wvc[:], wv_d[:, :, ch * 512:(ch + 1) * 512])
                for tt in range(4):
                    tsl = slice(tok0 + tt * 128, tok0 + (tt + 1) * 128)
                    ps = po["psum_pr"].tile([128, 512], F32, tag="proj")
                    for k2 in range(KT // 2):
                        nc.tensor.matmul(
                            ps[:], h3[:, 2 * k2:2 * k2 + 2, tsl],
                            wvc[:, 2 * k2:2 * k2 + 2, :],
                            start=(k2 == 0), stop=(k2 == KT // 2 - 1),
                            perf_mode=DR)
                        if k2 == KT // 4:
                            yield
                    sub = sub0 + tt
                    nc.vector.tensor_copy(
                        v_sb[:, ch * 8:(ch + 1) * 8, sub, 0:64],
                        ps[:].rearrange("p (h e) -> p h e", e=64))
                    yield

        # ---- cross-attn K projection (generator, from hT stream) --------
        def gen_ca_k():
            for hch in range(M // 512):
                hsl = slice(hch * 512, (hch + 1) * 512)
                hTc = po["strm"].tile([128, KT, 512], F8, tag="hTc")
                nc.sync.dma_start(hTc[:], hT_d[:, :, hsl])
                for ot in range(8):
                    wst = po["w"].tile([128, KT, 128], F8, tag="wst8q")
                    nc.sync.dma_start(wst[:], wkca_d[:, ot])
                    ps = po["psum_pr"].tile([128, 512], F32, tag="proj")
                    for k2 in range(KT // 2):
                        nc.tensor.matmul(
                            ps[:], wst[:, 2 * k2:2 * k2 + 2, :],
                            hTc[:, 2 * k2:2 * k2 + 2, :],
                            start=(k2 == 0), stop=(k2 == KT // 2 - 1),
                            perf_mode=DR)
                        if k2 == 1:
                            yield
                    stage_to_dram(ps, k_c[:, ot, hsl])
                    yield

        # ---- cross-attn V projection (generator, from hT stream) --------
        def gen_ca_v(b, sub0):
            for hch in range(2):          # two 512-token chunks per batch
                tok0 = b * S + hch * 512
                hsl = slice(tok0, tok0 + 512)
                hTc = po["strm"].tile([128, KT, 512], F8, tag="hTc")
                nc.sync.dma_start(hTc[:], hT_d[:, :, hsl])
                yield from gen_v_proj(hTc, wvca_d, sub0 + 4 * hch, 0)

        # ---- attention (generator) --------------------------------------
        def gen_attention(q_dr, k_dr, sub0, s_len, b, st):
            """Attention for batch b: q/k strips from DRAM, v from v_sb subs
            [sub0, sub0 + s_len/128). Scores e/o row-paired; exp [128,1024]
            f32->f8; PV fp8 DoubleRow over 2 s-chunks. Unnormalized ctx (f8)
            and denominators are collected into `st`; normalization happens
            in gen_att_norm (scheduled as a filler of the next phase)."""
            n_s = s_len // 128
            bsl = slice(b * T, (b + 1) * T)
            rD = po["att"].tile([16, 512], BF16, tag="rD", bufs=2)
            cs = []
            st["rD"] = rD
            st["cs"] = cs
            for hp in range(H // 2):
                qp = po["strm"].tile([128, 512], BF16, tag="qp")
                nc.sync.dma_start(qp[:], q_dr[:, hp, bsl])
                kp = po["strm"].tile([128, 1024], BF16, tag="kp")
                nc.sync.dma_start(kp[:, 0:s_len],
                                  k_dr[:, hp, b * s_len:(b + 1) * s_len])
                ctx_e = po["psum_ctx"].tile([65, 512], F32, tag="ctx")
                ctx_o = po["psum_ctx"].tile([65, 512], F32, tag="ctx")
                h0 = hp * 2
                for c2 in range(n_s // 2):
                    sc_e = po["psum_sc"].tile([128, 1024], F32, tag="sc")
                    sc_o = po["psum_sc"].tile([128, 1024], F32, tag="sc")
                    for j in range(2):
                        ssl = slice((2 * c2 + j) * 128, (2 * c2 + j + 1) * 128)
                        osl = slice(j * 512, (j + 1) * 512)
                        # paired: rows 0-63 and 64-127 run concurrently
                        nc.tensor.matmul(sc_e[:, osl], kp[0:64, ssl],
                                         qp[0:64, :], start=True, stop=True)
                        nc.tensor.matmul(sc_o[:, osl], kp[64:128, ssl],
                                         qp[64:128, :], start=True, stop=True)
                    e_e = po["e8"].tile([128, 2, 512], F8, tag="e")
                    e_o = po["e8"].tile([128, 2, 512], F8, tag="e")
                    nc.scalar.activation(
                        e_e[:].rearrange("p a t -> p (a t)"),
                        sc_e[:], AF.Exp, scale=EXP_SC)
                    nc.scalar.activation(
                        e_o[:].rearrange("p a t -> p (a t)"),
                        sc_o[:], AF.Exp, scale=EXP_SC)
                    yield
                    st = (c2 == 0)
                    sp = (c2 == n_s // 2 - 1)
                    sub = sub0 + 2 * c2
                    nc.tensor.matmul(
                        ctx_e[:], v_sb[:, h0, sub:sub + 2, 0:65],
                        e_e[:], start=st, stop=sp, perf_mode=DR)
                    nc.tensor.matmul(
                        ctx_o[:], v_sb[:, h0 + 1, sub:sub + 2, 0:65],
                        e_o[:], start=st, stop=sp, perf_mode=DR)
                    yield
                # epilogue: drain unnormalized ctx (f8, x WSC_V) + denom rows
                # (bf16) so the psum banks free quickly; normalization happens
                # after the batched reciprocal below.
                cs_e = po["csb"].tile([64, 512], F8, tag="cse")
                cs_o = po["csb"].tile([64, 512], F8, tag="cso")
                dn = po["work"].tile([65, 512], BF16, tag="dn")
                nc.vector.tensor_copy(cs_e[:], ctx_e[0:64, :])
                nc.vector.tensor_copy(dn[64:65, :], ctx_e[64:65, :])
                nc.gpsimd.dma_start(rD[2 * hp:2 * hp + 1, :], dn[64:65, :])
                yield
                dn2 = po["work"].tile([65, 512], BF16, tag="dn")
                nc.vector.tensor_copy(cs_o[:], ctx_o[0:64, :])
                nc.vector.tensor_copy(dn2[64:65, :], ctx_o[64:65, :])
                nc.gpsimd.dma_start(rD[2 * hp + 1:2 * hp + 2, :],
                                    dn2[64:65, :])
                cs.append((cs_e, cs_o))
                yield

        def gen_att_norm(st, b):
            """Normalize collected ctx by the batched softmax reciprocals and
            write ctxT. Runs as a filler of the phase after the attention."""
            bsl = slice(b * T, (b + 1) * T)
            rD = st["rD"]
            cs = st["cs"]
            rDf = po["att"].tile([16, 512], F32, tag="rDf")
            nc.vector.tensor_copy(rDf[:], rD[:])
            rI = po["att"].tile([16, 512], F32, tag="rI")
            nc.vector.reciprocal_approx_fast(rI[:], rDf[:])
            rI16 = po["att"].tile([16, 512], BF16, tag="rI16")
            nc.vector.tensor_scalar_mul(rI16[:], rI[:], RNORM)
            yield
            for hp in range(H // 2):
                cs_e, cs_o = cs[hp]
                # broadcast the pair's reciprocals across partitions with a
                # rank-16 PE matmul: rows 0-63 <- rI16[2hp], 64-127 <- [2hp+1]
                rb = po["psum_sc"].tile([128, 1024], F32, tag="sc")
                nc.tensor.matmul(rb[:, 0:512], sel_sb[:, hp, :], rI16[:],
                                 start=True, stop=True)
                nc.vector.tensor_tensor(ctxT[0:64, hp, bsl], cs_e[:],
                                        rb[0:64, 0:512], ALU.mult)
                yield
                todd = po["work"].tile([64, 512], F8, tag="todd")
                nc.vector.tensor_tensor(todd[:], cs_o[:], rb[64:128, 0:512],
                                        ALU.mult)
                nc.gpsimd.dma_start(ctxT[64:128, hp, bsl], todd[:])
                yield

        # ---- out-projection (generator) ---------------------------------
        def gen_out_proj(w_d, b):
            tsl = slice(b * 512, (b + 1) * 512)

            def cb(ot, ps, _tsl=tsl):
                nc.vector.scalar_tensor_tensor(
                    xbuf[:, ot, _tsl], ps[:], IWSC, xbuf[:, ot, _tsl],
                    ALU.mult, ALU.add)
            yield from gen_fm_proj(w_d, 8, KT, ctxT, tsl, cb, "wst8q",
                                   dr=True)

        # ---- qkv for self-attention (generator) -------------------------
        def gen_sa_qkv():
            # q/k psum comes from the (idle in P1) score pool so the q/k and
            # v projections rotate through independent psum slots
            for bch in range(2):
                tsl = slice(bch * 512, (bch + 1) * 512)

                def qk_cb(ot, ps, _tsl=tsl):
                    if ot < 8:
                        stage_to_dram(ps, q_s[:, ot, _tsl])
                    else:
                        stage_to_dram(ps, k_s[:, ot - 8, _tsl])
                yield from gen_fm_proj(wqk_d, 16, KT, h8, tsl, qk_cb,
                                       "wst8q", dr=True, psp="psum_sc",
                                       pst="sc")
            for b in range(2):
                yield from gen_v_proj(h8, wvsa_d, 4 * b, b * 512)

        # ---- cross-attn q projection (generator) ------------------------
        def gen_ca_q(b):
            tsl = slice(b * 512, (b + 1) * 512)

            def q2_cb(ot, ps, _tsl=tsl):
                stage_to_dram(ps, q_c[:, ot, _tsl])
            yield from gen_fm_proj(wqca_d, 8, KT, h8, tsl, q2_cb, "wst8q",
                                   dr=True)

        # ---- MLP (generator, one 512-token batch chunk) ------------------
        def gen_mlp(b):
            tsl = slice(b * 512, (b + 1) * 512)

            def fc_cb(ot, ps):
                # psum = fc_true/2 (wfc halved on host); quickgelu(x) ==
                # (1+tanh(0.851x)) * x/2, and tanh lives in exp_and_others.
                th = po["work"].tile([128, 512], BF16, tag="th")
                nc.scalar.activation(th[:], ps[:], AF.Tanh, scale=GELU_A)
                nc.vector.scalar_tensor_tensor(gbuf[:, ot], th[:], 1.0,
                                               ps[:], ALU.add, ALU.mult)
            yield from gen_fm_proj(wfc_d, 32, KT, hbuf, slice(0, 512),
                                   fc_cb, "wst16")

            def proj_cb(ot, ps, _tsl=tsl):
                nc.vector.tensor_tensor(xbuf[:, ot, _tsl], ps[:],
                                        xbuf[:, ot, _tsl], ALU.add)
                nc.sync.dma_start(outT_d[:, ot, _tsl], xbuf[:, ot, _tsl])
            yield from gen_fm_proj(wproj_d, 8, 32, gbuf, slice(0, 512),
                                   proj_cb, "wbig", pool="wb", wchunk=16)

        # ---- weave: out-projection + following LN casts/stats -----------
        def gen_proj_then_ln(w_d, b, ln):
            """Out-proj drains feed the next LN: after each pair of ots the
            LN casts that k2 (reads the freshly-updated xbuf); ends with the
            LN stats burst + rstd chain. Caller resumes `ln` for the apply."""
            ots = 0
            for v in gen_out_proj(w_d, b):
                yield v
                if v == "ot":
                    ots += 1
                    if ots % 2 == 0:
                        next(ln)
                        yield
            next(ln)
            yield

        # =================== schedule ====================================
        cak = gen_ca_k()
        for _ in range(3):
            next(cak)       # prefetch hT chunk 0 + first K weights first

        # x loads after the cak prefetch; per-kt so LN1's casts start early
        for ch in range(N // 512):
            sl = slice(ch * 512, (ch + 1) * 512)
            for kt in range(KT):
                nc.sync.dma_start(xbuf[:, kt, sl], xT_d[:, kt, sl])

        # P0: LN1 zipped with cross-K (independent, fills the LN ramp)
        _drive_until(gen_ln(slice(0, 512), True), cak)
        _drive_until(gen_ln(slice(512, 1024), True), cak)

        # P1: SA qkv (dense; keep cak for the attention phases)
        _drive_until(gen_sa_qkv())

        # P2: SA attention b0; zipped with cak + CA-V(b0) into subs 8-15
        cav0 = gen_ca_v(0, 8)
        st_s0, st_s1, st_c0, st_c1 = {}, {}, {}, {}
        _drive_until(gen_attention(q_s, k_s, 0, T, 0, st_s0), cav0, cak)

        # P3: SA attention b1; zipped with
        # norm(SA b0)+SAout(b0)+LN2(b0)+CAq(b0) + rest
        def gen_tail0():
            yield from gen_att_norm(st_s0, 0)
            ln = gen_ln(slice(0, 512), True)
            yield from gen_proj_then_ln(wosa_d, 0, ln)
            yield from ln
            yield from gen_ca_q(0)
        tail0 = gen_tail0()
        _drive_until(gen_attention(q_s, k_s, 4, T, 1, st_s1), tail0, cav0,
                     cak)

        # P4: CA attention b0 (subs 8-15); zipped with
        # norm(SA b1)+SAout(b1)+LN2(b1)+CAq(b1) and CA-V(b1) into subs 0-7
        _drain(tail0, cav0, cak)

        def gen_tail1():
            yield from gen_att_norm(st_s1, 1)
            ln = gen_ln(slice(512, 1024), True)
            yield from gen_proj_then_ln(wosa_d, 1, ln)
            yield from ln
            yield from gen_ca_q(1)
        tail1 = gen_tail1()
        cav1 = gen_ca_v(1, 0)
        _drive_until(gen_attention(q_c, k_c, 8, S, 0, st_c0), tail1, cav1)

        # P5: CA attention b1 (subs 0-7); zipped with
        # norm(CA b0)+CAout(b0)+LN3(b0)+MLP(b0)
        _drain(tail1, cav1)

        ln3b0_flag = {"stats_emitted": False}

        def gen_tail2():
            yield from gen_att_norm(st_c0, 0)
            ln = gen_ln(slice(0, 512), False, ln3b0_flag)
            yield from gen_proj_then_ln(wo_ca_d, 0, ln)
            yield from ln
            yield from gen_mlp(0)
        tail2 = gen_tail2()
        _drive_until(gen_attention(q_c, k_c, 0, S, 1, st_c1), _slow(tail2, 2))

        # P6: finish MLP(b0), zipped with norm(CA b1)+CAout(b1), then emit
        # LN3(b1)'s casts+stats+rstd as soon as LN3(b0)'s stats have consumed
        # the shared xball tiles (the hbuf/gbuf-touching LN3 apply + MLP(b1)
        # wait for P7).
        ln3b = gen_ln(slice(512, 1024), False)

        def gen_tail3a():
            yield from gen_att_norm(st_c1, 1)
            yield from gen_out_proj(wo_ca_d, 1)
            while not ln3b0_flag["stats_emitted"]:
                yield                   # tail2 (the primary) will get there
            for _ in range(KT // 2 + 1):
                next(ln3b)              # casts + dense stats + rstd chain
                yield
        tail3a = gen_tail3a()
        _drive_until(tail2, tail3a)
        _drain(tail3a)

        # P7: LN3(b1) apply + MLP(b1)  (dense tail)
        def gen_tail3b():
            yield from ln3b
            yield from gen_mlp(1)
        _drive_until(gen_tail3b())

    nc.compile()
    return nc


# ---------------------------------------------------------------------------
# host side
# ---------------------------------------------------------------------------

def _tile4(w):
    """[Din, Dout] -> [128, Dout/128, Din/128, 128] (p, ot, kt, o)."""
    din, dout = w.shape
    return np.ascontiguousarray(
        w.reshape(din // 128, 128, dout // 128, 128).transpose(1, 2, 0, 3))


def _rhs_tiled(w):
    """[Din, Dout] -> [128, Din/128, Dout] (p, kt, o)."""
    din, dout = w.shape
    return np.ascontiguousarray(
        w.reshape(din // 128, 128, dout).transpose(1, 0, 2))


def _prep_host(inputs):
    f32 = np.float32
    g = {k: np.asarray(v, f32) for k, v in inputs.items()}
    x, hs = g["x"], g["hidden_states"]
    scale = f32(1.0 / np.sqrt(HD))

    wq, wk, wv = np.split(g["sa_in_w"], 3, axis=0)
    bq, bk, bv = np.split(g["sa_in_b"], 3)
    wq_e = (wq * g["ln1_g"][None, :]) * scale
    bq_e = (wq @ g["ln1_b"]) * scale + bq
    wk_e = wk * g["ln1_g"][None, :]
    bk_e = wk @ g["ln1_b"] + bk
    wv_e = wv * g["ln1_g"][None, :]
    bv_e = wv @ g["ln1_b"] + bv

    cq, ck, cv = np.split(g["ca_in_w"], 3, axis=0)
    cbq, cbk, cbv = np.split(g["ca_in_b"], 3)
    cq_e = (cq * g["ln2_g"][None, :]) * scale
    cbq_e = (cq @ g["ln2_b"]) * scale + cbq
    # k/v of cross-attn apply to raw hidden_states: no LN fold
    fc_e = g["fc_w"] * g["ln3_g"][None, :]
    fcb_e = g["fc_w"] @ g["ln3_b"] + g["fc_b"]

    nz = lambda a: bool(np.abs(a).max() > 0)
    assert not any(nz(a) for a in
                   (bq_e, bk_e, bv_e, g["sa_out_b"], cbq_e, cbk, cbv,
                    g["ca_out_b"], fcb_e, g["proj_b"])), \
        "kernel compiled for the zero-bias configuration"

    wqk = np.concatenate([wq_e, wk_e], axis=0)     # [2D, D]

    bf = lambda a: np.ascontiguousarray(a.astype(BF16NP))
    f8 = lambda a, s=WSC: np.ascontiguousarray(
        (a * np.float32(s)).astype(F8NP))
    weights = {
        "wqk": f8(_tile4(wqk.T)),
        "wvsa": f8(_rhs_tiled(wv_e.T), WSC_V),
        "wosa": f8(_tile4(g["sa_out_w"].T)),
        "wqca": f8(_tile4(cq_e.T)),
        "wkca": f8(_tile4(ck.T)),
        "wvca": f8(_rhs_tiled(cv.T), WSC_V),
        "woca": f8(_tile4(g["ca_out_w"].T)),
        "wfc": bf(_tile4(fc_e.T) * np.float32(0.5)),
        "wproj": bf(_tile4(g["proj_w"].T)),
    }
    sel = np.zeros((16, 8, 128), f32)
    for hp in range(8):
        sel[2 * hp, hp, 0:64] = 1.0
        sel[2 * hp + 1, hp, 64:128] = 1.0
    weights["sel"] = bf(sel)

    in_maps = []
    for c in range(NCORES):
        xs = x[:, 2 * c:2 * c + 2, :]              # [T, 2, D]
        xt = xs.transpose(2, 1, 0).reshape(KT, 128, N).transpose(1, 0, 2)
        hss = hs[:, 2 * c:2 * c + 2, :]
        ht = hss.transpose(2, 1, 0).reshape(KT, 128, M).transpose(1, 0, 2)
        im = dict(weights)
        im["xT"] = np.ascontiguousarray(xt.astype(f32))
        im["hT"] = np.ascontiguousarray(ht.astype(F8NP))
        in_maps.append(im)
    return in_maps


def _unshard(results):
    out = np.empty((T, B, D), np.float32)
    for c in range(NCORES):
        r = np.asarray(results[c]["outT"])         # [128, KT, N]
        arr = r.transpose(1, 0, 2).reshape(D, BPC, T)
        out[:, 2 * c:2 * c + 2, :] = arr.transpose(2, 1, 0)
    return out


_cache = {}


def _get_program():
    if "nc" not in _cache:
        _cache["nc"] = build_program()
    return _cache["nc"]


def kernel(**inputs):
    in_maps = _prep_host(inputs)
    nc = _get_program()
    res = bass_utils.run_bass_kernel_spmd(nc, in_maps,
                                          core_ids=list(range(NCORES)))
    return _unshard(res.results)


def kernel_traced(**inputs):
    """Like kernel() but with NTFF profiling; returns (out, exec_time_ns)."""
    import types
    import antenv  # noqa: F401
    if "antenv.axon_hooks" not in sys.modules:
        hooks = types.ModuleType("antenv.axon_hooks")
        hooks._hook = None
        hooks.set_axon_ntff_profile_hook = lambda h: setattr(hooks, "_hook", h)
        hooks.get_axon_ntff_profile_hook = lambda: hooks._hook
        sys.modules["antenv.axon_hooks"] = hooks
        try:
            import trn_agent_boot.trn_boot as _tb
            hooks._hook = _tb._ntff_profile_via_ctypes("/opt/axon/libaxon_pjrt.so")
        except Exception as e:  # pragma: no cover
            print("ntff hook unavailable:", e)
    in_maps = _prep_host(inputs)
    nc = _get_program()
    res = bass_utils.run_bass_kernel_spmd(nc, in_maps,
                                          core_ids=list(range(NCORES)),
                                          trace=True)
    return _unshard(res.results), res.exec_time_ns
